# revision 36
# baseline (speedup 1.0000x reference)
"""GAU (Gated Attention Unit) forward on 8 Trainium2 NeuronCores.

Data-parallel over batch: B=32 -> 4 batch elements per core, every core runs
the identical program on its batch shard with full (replicated) weights.

Mixed-precision schedule (error budget 2e-2 rel, validated by simulation):
  - LayerNorm in fp32 (DVE), xn stored bf16 pre-scaled by S_X.
  - base/qk path: bf16 matmuls (PE transpose in bf16 at 1 cycle/row).
  - u, v and o projections: fp8(e4m3) DoubleRow matmuls in a 3-pass
    "2-word x 2-word" scheme: X = Xhi + Xlo, W = Whi + Wlo (same power-of-2
    scale for both words), computing Whi@Xhi (chunk-paired) plus one
    DoubleRow per chunk packing (Xlo@Whi + Xhi@Wlo).  0.75x the f32r PE
    cycles with ~bf16-class accuracy.
  - attention kernel @ v: 2-pass DoubleRow (kern 2-word, v 1-word): 0.5x.
  - scores (q@k^T) stay f32r (same PE cost as fp8 there, zero error).
Engine balance: Activation keeps the silus (one act-table; the LN Sqrt is
batched per element to avoid table reloads), pure fp8/bf16 copies are split
across Act/gpsimd, fused scalar ops + residuals on DVE.  All runtime scale
factors are folded into existing fused ops (no standalone rescale passes).

Two phases per core: phase 1 (v/base weights resident, u-weights preloaded)
runs LN -> bf16 PE-transpose -> fp8 word split -> base/rope/scores/kern ->
v-projection, holding v and kern fp8 tiles in SBUF across the phase
boundary (only the xn fp8 words spill to DRAM).  Phase 2 fuses
u-projection + attention@v + gating per e-chunk -- the attention output is
consumed straight out of PSUM (no spill round-trip) -- then the
o-projection with residual add.
"""

import numpy as np
import ml_dtypes
from contextlib import ExitStack

import concourse.bass as bass
import concourse.tile as tile
from concourse import bacc, mybir
from concourse.bass_utils import run_bass_kernel_spmd
from concourse.masks import make_identity

F32 = mybir.dt.float32
F32R = mybir.dt.float32r
BF16 = mybir.dt.bfloat16
FP16 = mybir.dt.float16
F8 = mybir.dt.float8e4
DR = mybir.MatmulPerfMode.DoubleRow
AF = mybir.ActivationFunctionType
OP = mybir.AluOpType
E4NP = ml_dtypes.float8_e4m3
BFNP = ml_dtypes.bfloat16

B, T, H, E, S, L = 32, 512, 1024, 2048, 128, 512
NCORES = 8
BPC = B // NCORES          # batch elements per core
EPS = 1e-5
HC = H // 128              # 8 h-chunks (contraction chunks for projections)
EC = E // 128              # 16 e-chunks
TC = T // 128              # 4 token chunks

# fp8 scales (power-of-2; amax checked against e4m3 max 240 with margin)
S_X = 16.0        # xn (amax ~5.5 -> 88); xn stored pre-scaled
S_WUV = 1024.0    # uv_w eff (amax ~0.11 -> 111)
S_K = 2.0         # attention kernel (amax ~34 -> 68); folded into q + bias
S_G = 0.5         # gate g = u*attn (amax ~195 -> 98)
S_WO = 1024.0     # o_w (amax ~0.11 -> 111)


def _emit(nc, with_vbias):
    x_d = nc.dram_tensor("x_in", [BPC, T, H], F32, kind="ExternalInput")
    xb_d = nc.dram_tensor("xb_in", [BPC, T, H], BF16, kind="ExternalInput")
    wv2_d = nc.dram_tensor("wv2_in", [HC, 128, 2, E], F8, kind="ExternalInput")
    wb_d = nc.dram_tensor("wb_in", [HC, 128, S], BF16, kind="ExternalInput")
    wu2_d = nc.dram_tensor("wu2_in", [HC, 128, 2, E], F8, kind="ExternalInput")
    wo2_d = nc.dram_tensor("wo2_in", [EC, 128, 2, H], F8, kind="ExternalInput")
    biasT_d = nc.dram_tensor("biasT_in", [T, T], BF16, kind="ExternalInput")
    ropeC_d = nc.dram_tensor("ropeC_in", [S, T], BF16, kind="ExternalInput")
    ropeS_d = nc.dram_tensor("ropeS_in", [S, T], BF16, kind="ExternalInput")
    gb_d = nc.dram_tensor("gb_in", [S, 4], F32, kind="ExternalInput")
    ubu_d = nc.dram_tensor("ubu_in", [128, EC], F32, kind="ExternalInput")
    ubb_d = nc.dram_tensor("ubb_in", [S, 1], F32, kind="ExternalInput")
    vb_d = nc.dram_tensor("vb_in", [1, E], F32R, kind="ExternalInput")
    y_d = nc.dram_tensor("y_out", [BPC, T, H], F32, kind="ExternalOutput")

    C_UV = 1.0 / (S_X * S_WUV)   # uv projection PSUM descale
    C_O = 1.0 / (S_G * S_WO)     # o projection PSUM descale
    # aT tiles carry S_K * attn; descale rides the gate stt scalar
    S_GK = S_G / S_K

    with tile.TileContext(nc) as tc, ExitStack() as ctx:
        consts = ctx.enter_context(tc.tile_pool(name="consts", bufs=1))
        holdp = ctx.enter_context(tc.tile_pool(name="holdp", bufs=1))
        p12w = ctx.enter_context(tc.tile_pool(name="p12w", bufs=1))
        vp = ctx.enter_context(tc.tile_pool(name="vp", bufs=2 * BPC))
        ktp = ctx.enter_context(tc.tile_pool(name="ktp", bufs=2 * BPC))
        dram = ctx.enter_context(tc.tile_pool(name="dram", bufs=1, space="DRAM"))
        # xn8 spill: fp8 word planes, [lo, hi] order
        xn8_spill = dram.tile([BPC, HC, 2, 128, T], F8)

        # ================= PHASE 1 =================
        with ExitStack() as p1:
            xp = p1.enter_context(tc.tile_pool(name="xp", bufs=2 * TC))
            xnp = p1.enter_context(tc.tile_pool(name="xnp", bufs=TC + 1))
            p1w = p1.enter_context(tc.tile_pool(name="p1w", bufs=1))
            xnTp = p1.enter_context(tc.tile_pool(name="xnTp", bufs=2))
            xn8p = p1.enter_context(tc.tile_pool(name="xn8p", bufs=2))
            rw = p1.enter_context(tc.tile_pool(name="rw", bufs=2))
            p1c = p1.enter_context(tc.tile_pool(name="p1c", bufs=1))
            ps = p1.enter_context(tc.tile_pool(name="ps", bufs=4, space="PSUM"))
            psa = p1.enter_context(tc.tile_pool(name="psa", bufs=2, space="PSUM"))
            pstp = p1.enter_context(tc.tile_pool(name="pstp", bufs=2,
                                                 space="PSUM"))

            # sqrt computes std/S_X directly: sqrt(var/S_X^2 + eps/S_X^2)
            eps_t = p1c.tile([128, 1], F32, tag="eps")
            nc.vector.memset(eps_t, EPS / (S_X * S_X))
            warm = p1c.tile([128, 1], F32, tag="warm")
            nc.scalar.activation(out=warm[:], in_=eps_t[:], func=AF.Silu)
            nc.scalar.activation(out=warm[:], in_=warm[:], func=AF.Sqrt)

            def load_x_ln(e):
                """Load x tiles, LayerNorm; xn stored bf16 scaled by S_X.
                e==0 normalizes per tile (fastest start); later elements
                batch the Sqrt so act-table swaps stay rare."""
                batched = e > 0
                xts = []
                mv4 = rw.tile([128, 2, TC], F32, tag="mv4", bufs=2)
                xn_tiles = []
                for tci in range(TC):
                    xt = xp.tile([128, H], BF16, tag="x")
                    nc.sync.dma_start(
                        out=xt, in_=xb_d[e, tci * 128:(tci + 1) * 128, :])
                    st = rw.tile([128, 2, 6], F32, tag="bnst", bufs=4)
                    xv = xt[:].rearrange("p (g d) -> p g d", g=2)
                    nc.vector.bn_stats(out=st[:, 0, :], in_=xv[:, 0, :])
                    nc.vector.bn_stats(out=st[:, 1, :], in_=xv[:, 1, :])
                    nc.vector.bn_aggr(out=mv4[:, :, tci], in_=st[:])
                    xts.append(xt)
                    if not batched:
                        sstd1 = rw.tile([128, 1], F32, tag="sstd1", bufs=4)
                        nc.scalar.activation(
                            out=sstd1[:], in_=mv4[:, 1, tci:tci + 1],
                            func=AF.Sqrt, bias=eps_t[:],
                            scale=1.0 / (S_X * S_X))
                        rstd1 = rw.tile([128, 1], F32, tag="rstd1", bufs=4)
                        nc.vector.reciprocal(out=rstd1[:], in_=sstd1[:])
                        xn = xnp.tile([128, H], BF16, tag="xn")
                        nc.vector.tensor_scalar(
                            out=xn[:], in0=xt[:],
                            scalar1=mv4[:, 0, tci:tci + 1],
                            scalar2=rstd1[:],
                            op0=OP.subtract, op1=OP.mult)
                        xn_tiles.append(xn)
                if batched:
                    sstd = rw.tile([128, TC], F32, tag="sstd", bufs=2)
                    nc.scalar.activation(
                        out=sstd[:], in_=mv4[:, 1, :], func=AF.Sqrt,
                        bias=eps_t[:], scale=1.0 / (S_X * S_X))
                    rstd = rw.tile([128, TC], F32, tag="rstd", bufs=2)
                    nc.vector.reciprocal(out=rstd[:], in_=sstd[:])
                    for tci in range(TC):
                        xn = xnp.tile([128, H], BF16, tag="xn")
                        nc.vector.tensor_scalar(
                            out=xn[:], in0=xts[tci][:],
                            scalar1=mv4[:, 0, tci:tci + 1],
                            scalar2=rstd[:, tci:tci + 1],
                            op0=OP.subtract, op1=OP.mult)
                        xn_tiles.append(xn)
                return xn_tiles

            def transpose_xn(e, xn_tiles):
                """PE-transpose (bf16) then hi/lo fp8 words (single-op each).
                xnT carries S_X*xn; word order [lo, hi]."""
                xnT = xnTp.tile([128, HC, T], BF16, tag="xnT")
                if e == 0:
                    xn8 = holdp.tile([128, HC, 2, T], F8, tag="xn8_hold",
                                     name="xn8_hold")
                else:
                    xn8 = xn8p.tile([128, HC, 2, T], F8, tag="xn8")
                for tci in range(TC):
                    for hcg in range(2):
                        tps = pstp.tile([128, 4, 128], BF16, tag="tps",
                                        name="tps")
                        for j in range(4):
                            hc = hcg * 4 + j
                            nc.tensor.transpose(
                                tps[:, j, :],
                                xn_tiles[tci][:, hc * 128:(hc + 1) * 128],
                                ident[:])
                        nc.any.tensor_copy(
                            out=xnT[:, hcg * 4:(hcg + 1) * 4,
                                    tci * 128:(tci + 1) * 128],
                            in_=tps[:])
                h4 = HC // 2
                nc.scalar.activation(
                    out=xn8[:, :h4, 1, :], in_=xnT[:, :h4, :], func=AF.Copy)
                nc.vector.tensor_tensor(
                    out=xn8[:, :h4, 0, :], in0=xnT[:, :h4, :],
                    in1=xn8[:, :h4, 1, :], op=OP.subtract)
                nc.gpsimd.tensor_copy(
                    out=xn8[:, h4:, 1, :], in_=xnT[:, h4:, :])
                nc.vector.tensor_tensor(
                    out=xn8[:, h4:, 0, :], in0=xnT[:, h4:, :],
                    in1=xn8[:, h4:, 1, :], op=OP.subtract)
                if e != 0:
                    nc.sync.dma_start(
                        out=xn8_spill[e].rearrange("c j p t -> p c j t"),
                        in_=xn8[:])
                return xnT, xn8

            # ---- priority order: x(e0)+LN first, wv2 immediately after ----
            xn0 = load_x_ln(0)
            wv2 = p1w.tile([128, HC, 2, E], F8, tag="wv2", name="wv2")
            for k in range(HC):
                nc.sync.dma_start(out=wv2[:, k, :, :], in_=wv2_d[k])

            ident_f = p1c.tile([128, 128], F32, tag="ident_f")
            make_identity(nc, ident_f)
            ident = p1c.tile([128, 128], BF16, tag="ident")
            nc.vector.tensor_copy(out=ident[:], in_=ident_f[:])
            perm_f = p1c.tile([128, 128], F32, tag="perm_f")
            nc.gpsimd.memset(perm_f, 0.0)
            for base in (-64, 64):
                nc.gpsimd.affine_select(
                    out=perm_f, in_=perm_f, compare_op=OP.not_equal,
                    fill=1.0, base=base, pattern=[[-1, 128]],
                    channel_multiplier=1)
            perm = p1c.tile([128, 128], BF16, tag="perm")
            nc.vector.tensor_copy(out=perm[:], in_=perm_f[:])
            # base weights (bf16)
            wb = p1w.tile([128, HC, S], BF16, tag="wb", name="wb")
            nc.sync.dma_start(
                out=wb, in_=wb_d[:].rearrange("c p s -> p c s"))
            gb = p1c.tile([S, 4], F32, tag="gb")
            nc.sync.dma_start(out=gb, in_=gb_d[:])
            ubu = consts.tile([128, EC], F32, tag="ubu")
            nc.sync.dma_start(out=ubu, in_=ubu_d[:])
            ubb = p1c.tile([S, 1], F32, tag="ubb")
            nc.sync.dma_start(out=ubb, in_=ubb_d[:])
            if with_vbias:
                of = p1c.tile([1, 128], F32, tag="ones_row_f")
                nc.vector.memset(of, 1.0)
                ones_row = p1c.tile([1, 128], F32R, tag="ones_row")
                nc.vector.tensor_copy(out=ones_row[:], in_=of[:])
                vb_row = p1c.tile([1, E], F32R, tag="vb_row")
                nc.sync.dma_start(out=vb_row, in_=vb_d[:])

            ropeC = p1c.tile([S, T], BF16, tag="ropeC")
            nc.sync.dma_start(out=ropeC, in_=ropeC_d[:])
            ropeS = p1c.tile([S, T], BF16, tag="ropeS")
            nc.sync.dma_start(out=ropeS, in_=ropeS_d[:])
            biasT = p1c.tile([128, TC, T], BF16, tag="biasT")
            nc.sync.dma_start(
                out=biasT, in_=biasT_d[:].rearrange("(c p) i -> p c i", p=128))

            # u-projection weights preloaded during phase 1 (kills the
            # phase-boundary DMA bubble)
            wu2 = p12w.tile([128, HC, 2, E], F8, tag="wu2", name="wu2")
            for k in range(HC):
                nc.sync.dma_start(out=wu2[:, k, :, :], in_=wu2_d[k])

            def rope_pre(bps):
                # bps carries S_X * base_pre -> descale via silu scale
                ubT = rw.tile([S, T], BF16, tag="ubT")
                nc.scalar.activation(
                    out=ubT[:], in_=bps[:], func=AF.Silu, bias=ubb[:],
                    scale=1.0 / S_X)
                pres = []
                for qi in (0, 1):
                    pre = rw.tile([S, T], BF16, tag="pre")
                    nc.vector.tensor_scalar(
                        out=pre[:], in0=ubT[:],
                        scalar1=gb[:, 2 * qi:2 * qi + 1],
                        scalar2=gb[:, 2 * qi + 1:2 * qi + 2],
                        op0=OP.mult, op1=OP.add)
                    pres.append(pre)
                return pres

            def rope_finish(pres):
                qkts = []
                for pre in pres:
                    sps = ps.tile([128, T], F32, tag="ps", name="sps")
                    nc.tensor.matmul(sps[:], perm[:], pre[:],
                                     start=True, stop=True)
                    t1 = rw.tile([S, T], BF16, tag="t1")
                    nc.vector.tensor_tensor(
                        out=t1[:], in0=pre[:], in1=ropeC[:], op=OP.mult)
                    t2 = rw.tile([S, T], BF16, tag="t2")
                    nc.vector.tensor_tensor(
                        out=t2[:], in0=sps[:], in1=ropeS[:], op=OP.mult)
                    qkt = rw.tile([S, T], BF16, tag="qkt", bufs=2)
                    nc.vector.tensor_tensor(
                        out=qkt[:], in0=t1[:], in1=t2[:], op=OP.add)
                    qkts.append(qkt)
                return qkts

            def base_proj(xnT):
                bps = ps.tile([128, T], F32, tag="ps", name="bps")
                for k in range(HC):
                    nc.tensor.matmul(
                        bps[:], wb[:, k, :], xnT[:, k, :],
                        start=(k == 0), stop=(k == HC - 1))
                return bps

            def scores_kern(qT, kT):
                """q@k^T (f32r) then kern hi/lo fp8 (pair tiles [128,2,2,T])."""
                khl_tiles = [ktp.tile([128, 2, 2, T], F8, tag="khl",
                                      name="khl")
                             for _ in range(TC // 2)]
                for jcp in range(TC // 2):
                    kf = rw.tile([128, 2, T], BF16, tag="kernf", bufs=2)
                    for j in range(2):
                        jc = 2 * jcp + j
                        scps = ps.tile([128, T], F32, tag="ps", name="scps")
                        nc.tensor.matmul(
                            scps[:], kT[:, jc * 128:(jc + 1) * 128], qT[:],
                            start=True, stop=True)
                        tadd = rw.tile([128, T], F32, tag="tadd", bufs=2)
                        nc.vector.tensor_tensor(
                            out=tadd[:], in0=scps[:], in1=biasT[:, jc, :],
                            op=OP.add)
                        # relu(t)^2 == max(t,0)*t (t pre-scaled by sqrt(S_K))
                        nc.vector.scalar_tensor_tensor(
                            out=kf[:, j, :], in0=tadd[:], scalar=0.0,
                            in1=tadd[:], op0=OP.max, op1=OP.mult)
                    khl = khl_tiles[jcp]
                    nc.gpsimd.tensor_copy(out=khl[:, :, 0, :], in_=kf[:])
                    nc.vector.tensor_tensor(
                        out=khl[:, :, 1, :], in0=kf[:], in1=khl[:, :, 0, :],
                        op=OP.subtract)
                return khl_tiles

            def v_fin(v2_tiles, tci, fs, bank):
                if with_vbias:
                    nc.tensor.matmul(
                        bank[:], ones_row[:],
                        vb_row[:, fs * 512:(fs + 1) * 512],
                        start=False, stop=True)
                nc.scalar.activation(
                    out=v2_tiles[tci // 2][:, tci % 2,
                                           fs * 512:(fs + 1) * 512],
                    in_=bank[:], func=AF.Silu, scale=C_UV)

            def v_bank(xn8, tci, fs, bank):
                """3-pass fp8 DR accumulation for one [t128, col512] bank."""
                for c in range(0, HC, 2):
                    nc.tensor.matmul(
                        bank[:],
                        xn8[:, c:c + 2, 1, tci * 128:(tci + 1) * 128],
                        wv2[:, c:c + 2, 0, fs * 512:(fs + 1) * 512],
                        start=(c == 0), stop=False, perf_mode=DR)
                for c in range(HC):
                    nc.tensor.matmul(
                        bank[:],
                        xn8[:, c, 0:2, tci * 128:(tci + 1) * 128],
                        wv2[:, c, 0:2, fs * 512:(fs + 1) * 512],
                        start=False,
                        stop=(c == HC - 1) and not with_vbias,
                        perf_mode=DR)

            def v_proj(xn8):
                v2_tiles = [vp.tile([128, 2, E], F8, tag="v2", name="v2")
                            for _ in range(TC // 2)]
                pairs = [(tci, fs) for tci in range(TC)
                         for fs in range(E // 512)]
                for wi in range(0, len(pairs), 6):
                    wave = pairs[wi:wi + 6]
                    banks = {}
                    for i, p in enumerate(wave):
                        pool_ = psa if i < 2 else ps
                        banks[p] = pool_.tile([128, 512], F32,
                                              tag="psa" if i < 2 else "ps",
                                              name="vbank")
                    for (tci, fs) in wave:
                        v_bank(xn8, tci, fs, banks[(tci, fs)])
                    for (tci, fs) in wave:
                        v_fin(v2_tiles, tci, fs, banks[(tci, fs)])
                return v2_tiles

            xn8_hold = None
            xn_next = xn0
            kv_held = []
            for e in range(BPC):
                xn_e = xn_next
                xnT_e, xn8_e = transpose_xn(e, xn_e)
                if e == 0:
                    xn8_hold = xn8_e
                xn_next = load_x_ln(e + 1) if e + 1 < BPC else None
                bps = base_proj(xnT_e)
                pres = rope_pre(bps)
                qT, kT = rope_finish(pres)
                khl_e = scores_kern(qT, kT)
                v_e = v_proj(xn8_e)
                kv_held.append((khl_e, v_e))

        # ================= PHASE 2 =================
        # u-projection + attn@v + gating + o-projection, fused per element:
        # attn never leaves PSUM (no aT spill round-trip).
        with ExitStack() as p2:
            xn82p = p2.enter_context(tc.tile_pool(name="xn82p", bufs=2))
            p2w = p2.enter_context(tc.tile_pool(name="p2w", bufs=1))
            utp = p2.enter_context(tc.tile_pool(name="utp", bufs=2))
            gfp = p2.enter_context(tc.tile_pool(name="gfp", bufs=2))
            gtp = p2.enter_context(tc.tile_pool(name="gtp", bufs=EC))
            yp = p2.enter_context(tc.tile_pool(name="yp", bufs=2))
            xrp = p2.enter_context(tc.tile_pool(name="xrp", bufs=2))
            psu = p2.enter_context(tc.tile_pool(name="psu", bufs=3,
                                                space="PSUM"))
            psv = p2.enter_context(tc.tile_pool(name="psv", bufs=2,
                                                space="PSUM"))
            pso = p2.enter_context(tc.tile_pool(name="pso", bufs=3,
                                                space="PSUM"))

            def load_xn82(e):
                xn82 = xn82p.tile([128, HC, 2, T], F8, tag="xn82")
                nc.sync.dma_start(
                    out=xn82, in_=xn8_spill[e].rearrange("c j p t -> p c j t"))
                return xn82

            wo2 = [None] * (EC // 2)

            def load_wo_pair(ecp):
                woc = p2w.tile([128, 2, 2, H], F8, tag=f"wo{ecp}", name="woc")
                for i in range(2):
                    nc.sync.dma_start(
                        out=woc[:, i, :, :], in_=wo2_d[2 * ecp + i])
                wo2[ecp] = woc

            def u_bank(xn82, ec, bank):
                """3-pass fp8 DR for one [e128, T] u-projection bank."""
                for c in range(0, HC, 2):
                    nc.tensor.matmul(
                        bank[:],
                        wu2[:, c:c + 2, 0, ec * 128:(ec + 1) * 128],
                        xn82[:, c:c + 2, 1, :],
                        start=(c == 0), stop=False, perf_mode=DR)
                for c in range(HC):
                    nc.tensor.matmul(
                        bank[:],
                        wu2[:, c, 0:2, ec * 128:(ec + 1) * 128],
                        xn82[:, c, 0:2, :],
                        start=False, stop=(c == HC - 1), perf_mode=DR)

            def attnv_bank(khl_tiles, v2_tiles, ec, bank):
                """2-pass fp8 DR attention for one [e128, T] bank."""
                for w_ in range(2):       # 0 = hi planes, 1 = lo planes
                    for jcp in range(TC // 2):
                        nc.tensor.matmul(
                            bank[:],
                            v2_tiles[jcp][:, :, ec * 128:(ec + 1) * 128],
                            khl_tiles[jcp][:, :, w_, :],
                            start=(w_ == 0 and jcp == 0),
                            stop=(w_ == 1 and jcp == TC // 2 - 1),
                            perf_mode=DR)

            def u_attn_gate(e, xn82, khl_tiles, v2_tiles, chase_wo):
                """Per ec: u-proj bank + attnv bank, silu + gate from PSUM.
                g8 pair tiles [128, 2(ec), 2(lo/hi), T] fp8."""
                g8_tiles = [gtp.tile([128, 2, 2, T], F8, tag="g8", name="g8")
                            for _ in range(EC // 2)]
                gf = None
                for ec in range(EC):
                    ub = psu.tile([128, T], F32, tag="psu", name="ub")
                    u_bank(xn82, ec, ub)
                    ab = psv.tile([128, T], F32, tag="psv", name="ab")
                    attnv_bank(khl_tiles, v2_tiles, ec, ab)
                    if ec % 2 == 0:
                        gf = gfp.tile([128, 2, T], FP16, tag="gf")
                    ut = utp.tile([128, T], BF16, tag="uT")
                    nc.scalar.activation(
                        out=ut[:], in_=ub[:], func=AF.Silu,
                        bias=ubu[:, ec:ec + 1], scale=C_UV)
                    nc.vector.scalar_tensor_tensor(
                        out=gf[:, ec % 2, :], in0=ut[:], scalar=S_GK,
                        in1=ab[:], op0=OP.mult, op1=OP.mult)
                    if ec % 2 == 1:
                        ecp = ec // 2
                        g8 = g8_tiles[ecp]
                        nc.scalar.activation(
                            out=g8[:, :, 1, :], in_=gf[:], func=AF.Copy)
                        nc.vector.tensor_tensor(
                            out=g8[:, :, 0, :], in0=gf[:], in1=g8[:, :, 1, :],
                            op=OP.subtract)
                        if chase_wo and ecp < EC // 2:
                            load_wo_pair(ecp)
                return g8_tiles

            def o_bank(g8_tiles, tci, hs, bank):
                """3-pass fp8 DR for one [t128, h512] o-projection bank."""
                t0 = tci * 128
                h0 = hs * 512
                for ecp in range(EC // 2):
                    nc.tensor.matmul(
                        bank[:],
                        g8_tiles[ecp][:, :, 1, t0:t0 + 128],
                        wo2[ecp][:, :, 0, h0:h0 + 512],
                        start=(ecp == 0), stop=False, perf_mode=DR)
                for ec in range(EC):
                    nc.tensor.matmul(
                        bank[:],
                        g8_tiles[ec // 2][:, ec % 2, 0:2, t0:t0 + 128],
                        wo2[ec // 2][:, ec % 2, 0:2, h0:h0 + 512],
                        start=False, stop=(ec == EC - 1), perf_mode=DR)

            def o_fin(e, tci, hs, bank):
                xr = xrp.tile([128, 512], F32, tag="xr")
                nc.sync.dma_start(
                    out=xr,
                    in_=x_d[e, tci * 128:(tci + 1) * 128,
                            hs * 512:(hs + 1) * 512])
                yt = yp.tile([128, 512], F32, tag="y")
                nc.vector.scalar_tensor_tensor(
                    out=yt[:], in0=bank[:], scalar=C_O, in1=xr[:],
                    op0=OP.mult, op1=OP.add)
                nc.sync.dma_start(
                    out=y_d[e, tci * 128:(tci + 1) * 128,
                            hs * 512:(hs + 1) * 512],
                    in_=yt[:])

            def o_bank_half(g8_tiles, tci, hs, bank, half):
                """Half of the 3-pass accumulation (ec chunks split 2-way)."""
                t0 = tci * 128
                h0 = hs * 512
                ecps = range(half * (EC // 4), (half + 1) * (EC // 4))
                for i, ecp in enumerate(ecps):
                    nc.tensor.matmul(
                        bank[:],
                        g8_tiles[ecp][:, :, 1, t0:t0 + 128],
                        wo2[ecp][:, :, 0, h0:h0 + 512],
                        start=(i == 0), stop=False, perf_mode=DR)
                ecs = list(range(half * (EC // 2), (half + 1) * (EC // 2)))
                for i, ec in enumerate(ecs):
                    nc.tensor.matmul(
                        bank[:],
                        g8_tiles[ec // 2][:, ec % 2, 0:2, t0:t0 + 128],
                        wo2[ec // 2][:, ec % 2, 0:2, h0:h0 + 512],
                        start=False, stop=(i == len(ecs) - 1), perf_mode=DR)

            def o_fin2(e, tci, hs, b0, b1):
                xr = xrp.tile([128, 512], F32, tag="xr")
                nc.sync.dma_start(
                    out=xr,
                    in_=x_d[e, tci * 128:(tci + 1) * 128,
                            hs * 512:(hs + 1) * 512])
                tsum = yp.tile([128, 512], F32, tag="tsum")
                nc.vector.scalar_tensor_tensor(
                    out=tsum[:], in0=b0[:], scalar=C_O, in1=xr[:],
                    op0=OP.mult, op1=OP.add)
                yt = yp.tile([128, 512], F32, tag="y")
                nc.vector.scalar_tensor_tensor(
                    out=yt[:], in0=b1[:], scalar=C_O, in1=tsum[:],
                    op0=OP.mult, op1=OP.add)
                nc.sync.dma_start(
                    out=y_d[e, tci * 128:(tci + 1) * 128,
                            hs * 512:(hs + 1) * 512],
                    in_=yt[:])

            def o_proj(e, g8_tiles):
                pairs8 = [(tci, hs) for tci in range(TC)
                          for hs in range(H // 512)]
                if e == BPC - 1:
                    # tail: two parallel half-chains per output so the final
                    # drain is half as long; last output drains alone
                    tail_waves = [pairs8[0:2], pairs8[2:4], pairs8[4:6],
                                  pairs8[6:7], pairs8[7:8]]
                    for wave in tail_waves:
                        hb = {}
                        for j, p in enumerate(wave):
                            hb[p] = (pso.tile([128, 512], F32, tag="pso",
                                              name="ob0"),
                                     psu.tile([128, 512], F32, tag="psu",
                                              name="ob1"))
                        for (tci, hs) in wave:
                            o_bank_half(g8_tiles, tci, hs,
                                        hb[(tci, hs)][0], 0)
                            o_bank_half(g8_tiles, tci, hs,
                                        hb[(tci, hs)][1], 1)
                        for (tci, hs) in wave:
                            o_fin2(e, tci, hs, *hb[(tci, hs)])
                else:
                    for wi in range(0, len(pairs8), 3):
                        wave = pairs8[wi:wi + 3]
                        banks = {p: pso.tile([128, 512], F32, tag="pso",
                                             name="obank") for p in wave}
                        for (tci, hs) in wave:
                            o_bank(g8_tiles, tci, hs, banks[(tci, hs)])
                        for (tci, hs) in wave:
                            o_fin(e, tci, hs, banks[(tci, hs)])

            xn82_next = None
            for e in range(BPC):
                if e == 0:
                    xn82 = xn8_hold
                else:
                    xn82 = xn82_next
                khl_e, v_e = kv_held[e]
                g8_tiles = u_attn_gate(e, xn82, khl_e, v_e,
                                       chase_wo=(e == 0))
                if e + 1 < BPC:
                    xn82_next = load_xn82(e + 1)
                o_proj(e, g8_tiles)

    return nc


_BUILD_CACHE = {}


def _get_nc(with_vbias):
    key = bool(with_vbias)
    if key not in _BUILD_CACHE:
        nc = bacc.Bacc("TRN2", target_bir_lowering=False)
        _emit(nc, with_vbias)
        nc.compile()
        _BUILD_CACHE[key] = nc
    return _BUILD_CACHE[key]


def _rope_tables():
    """Rope sin/cos tables, computed with jax-on-cpu float32 ops exactly as
    the reference does (sin/cos of large fp32 arguments are implementation-
    sensitive, so matching op-for-op matters)."""
    import jax
    import jax.numpy as jnp

    cpu = jax.devices("cpu")[0]
    with jax.default_device(cpu):
        half = S // 2
        pos = jnp.arange(T, dtype=jnp.float32)
        inv_freq = 10000.0 ** (jnp.arange(half, dtype=jnp.float32) / half)
        sinusoid = pos[:, None] * inv_freq[None, :]          # [T, half]
        sin = np.asarray(jnp.sin(sinusoid)).astype(np.float32)
        cos = np.asarray(jnp.cos(sinusoid)).astype(np.float32)
    C = np.empty((S, T), np.float32)
    Sg = np.empty((S, T), np.float32)
    C[:half] = cos.T
    C[half:] = cos.T
    Sg[:half] = -sin.T   # q[s<64] = pre[s]*cos - pre[s+64]*sin
    Sg[half:] = sin.T    # q[s>=64] = pre[s]*cos + pre[s-64]*sin
    return C, Sg


def _split8(w, s):
    """2-word e4m3 split at common scale s: returns (hi, lo) planes."""
    hi = (w * s).astype(E4NP)
    lo = ((w * s).astype(np.float32) - hi.astype(np.float32)).astype(E4NP)
    return hi, lo


def _host_prep(x, ln_w, ln_b, uv_w, uv_b, gamma, beta, w, o_w, o_b):
    w_eff = uv_w * ln_w[None, :]                 # fold ln scale into weights
    uvb_eff = uv_b + uv_w @ ln_b                 # fold ln shift into biases
    uv_wT = np.ascontiguousarray(w_eff.T)        # [H, 2E+S]
    w_u = uv_wT[:, :E]
    w_v = uv_wT[:, E:2 * E]
    w_base = uv_wT[:, 2 * E:]
    wo = np.ascontiguousarray(o_w.T)             # [E, H]

    # fp8 hi/lo planes, [hi, lo] word order, chunked layouts
    vh, vl = _split8(w_v, S_WUV)
    wv2 = np.stack([vh.reshape(HC, 128, E), vl.reshape(HC, 128, E)], axis=2)
    uh, ul = _split8(w_u, S_WUV)
    wu2 = np.stack([uh.reshape(HC, 128, E), ul.reshape(HC, 128, E)], axis=2)
    oh, ol = _split8(wo, S_WO)
    wo2 = np.stack([oh.reshape(EC, 128, H), ol.reshape(EC, 128, H)], axis=2)

    wb = np.ascontiguousarray(w_base.reshape(HC, 128, S)).astype(BFNP)

    idx = np.arange(T)
    sqrt_sk = np.float32(np.sqrt(S_K))
    biasT = np.ascontiguousarray(
        w[idx[:, None] - idx[None, :] + (L - 1)] * sqrt_sk).astype(BFNP)

    ropeC, ropeS = _rope_tables()

    inv_sqrt_s = np.float32(1.0 / np.sqrt(np.float32(S)))
    # q side picks up 1/sqrt(S) and sqrt(S_K) so the DVE adds stay fused
    gb = np.stack([gamma[0] * inv_sqrt_s * sqrt_sk,
                   beta[0] * inv_sqrt_s * sqrt_sk,
                   gamma[1], beta[1]], axis=1).astype(np.float32)

    ubu = np.ascontiguousarray(
        uvb_eff[:E].reshape(EC, 128).T).astype(np.float32)
    ubb = uvb_eff[2 * E:].reshape(S, 1).astype(np.float32)
    vb = (uvb_eff[E:2 * E].reshape(1, E) * np.float32(S_X * S_WUV)
          ).astype(np.float32)
    return {
        "wv2_in": np.ascontiguousarray(wv2), "wb_in": wb,
        "wu2_in": np.ascontiguousarray(wu2),
        "wo2_in": np.ascontiguousarray(wo2), "biasT_in": biasT,
        "ropeC_in": ropeC.astype(BFNP), "ropeS_in": ropeS.astype(BFNP), "gb_in": gb,
        "ubu_in": ubu, "ubb_in": ubb, "vb_in": vb,
    }


def kernel(x, ln_w, ln_b, uv_w, uv_b, gamma, beta, w, o_w, o_b):
    x = np.ascontiguousarray(np.asarray(x, dtype=np.float32))
    args = [np.asarray(a, np.float32) for a in
            (ln_w, ln_b, uv_w, uv_b, gamma, beta, w, o_w, o_b)]
    ln_w, ln_b, uv_w, uv_b, gamma, beta, w, o_w, o_b = args

    shared = _host_prep(x, ln_w, ln_b, uv_w, uv_b, gamma, beta, w, o_w, o_b)
    with_vbias = bool(np.any(shared["vb_in"]))
    nc = _get_nc(with_vbias)

    in_maps = []
    for c in range(NCORES):
        m = dict(shared)
        xs = np.ascontiguousarray(x[c * BPC:(c + 1) * BPC])
        m["x_in"] = xs
        m["xb_in"] = xs.astype(BFNP)
        in_maps.append(m)

    res = run_bass_kernel_spmd(nc, in_maps, core_ids=list(range(NCORES)))
    out = np.concatenate([r["y_out"] for r in res.results], axis=0)
    if np.any(o_b):
        out = out + o_b[None, None, :]
    return out


# revision 37
# speedup vs baseline: 1.0186x; 1.0186x over previous
"""GAU (Gated Attention Unit) forward on 8 Trainium2 NeuronCores.

Data-parallel over batch: B=32 -> 4 batch elements per core, every core runs
the identical program on its batch shard with full (replicated) weights.

Mixed-precision schedule (error budget 2e-2 rel, validated by simulation):
  - LayerNorm in fp32 (DVE), xn stored bf16 pre-scaled by S_X.
  - base/qk path: bf16 matmuls (PE transpose in bf16 at 1 cycle/row).
  - u, v and o projections: fp8(e4m3) DoubleRow matmuls in a 3-pass
    "2-word x 2-word" scheme: X = Xhi + Xlo, W = Whi + Wlo (same power-of-2
    scale for both words), computing Whi@Xhi (chunk-paired) plus one
    DoubleRow per chunk packing (Xlo@Whi + Xhi@Wlo).  0.75x the f32r PE
    cycles with ~bf16-class accuracy.
  - attention kernel @ v: 2-pass DoubleRow (kern 2-word, v 1-word): 0.5x.
  - scores (q@k^T) stay f32r (same PE cost as fp8 there, zero error).
Engine balance: Activation keeps the silus (one act-table; the LN Sqrt is
batched per element to avoid table reloads), pure fp8/bf16 copies are split
across Act/gpsimd, fused scalar ops + residuals on DVE.  All runtime scale
factors are folded into existing fused ops (no standalone rescale passes).

Two phases per core: phase 1 (v/base weights resident, u-weights preloaded)
runs LN -> bf16 PE-transpose -> fp8 word split -> base/rope/scores/kern ->
v-projection, holding v and kern fp8 tiles in SBUF across the phase
boundary (only the xn fp8 words spill to DRAM).  Phase 2 fuses
u-projection + attention@v + gating per e-chunk -- the attention output is
consumed straight out of PSUM (no spill round-trip) -- then the
o-projection with residual add.
"""

import numpy as np
import ml_dtypes
from contextlib import ExitStack

import concourse.bass as bass
import concourse.tile as tile
from concourse import bacc, mybir
from concourse.bass_utils import run_bass_kernel_spmd
from concourse.masks import make_identity

F32 = mybir.dt.float32
F32R = mybir.dt.float32r
BF16 = mybir.dt.bfloat16
FP16 = mybir.dt.float16
F8 = mybir.dt.float8e4
DR = mybir.MatmulPerfMode.DoubleRow
AF = mybir.ActivationFunctionType
OP = mybir.AluOpType
E4NP = ml_dtypes.float8_e4m3
BFNP = ml_dtypes.bfloat16

B, T, H, E, S, L = 32, 512, 1024, 2048, 128, 512
NCORES = 8
BPC = B // NCORES          # batch elements per core
EPS = 1e-5
HC = H // 128              # 8 h-chunks (contraction chunks for projections)
EC = E // 128              # 16 e-chunks
TC = T // 128              # 4 token chunks

# fp8 scales (power-of-2; amax checked against e4m3 max 240 with margin)
S_X = 16.0        # xn (amax ~5.5 -> 88); xn stored pre-scaled
S_WUV = 1024.0    # uv_w eff (amax ~0.11 -> 111)
S_K = 2.0         # attention kernel (amax ~34 -> 68); folded into q + bias
S_G = 0.5         # gate g = u*attn (amax ~195 -> 98)
S_WO = 1024.0     # o_w (amax ~0.11 -> 111)


def _emit(nc, with_vbias):
    x_d = nc.dram_tensor("x_in", [BPC, T, H], F32, kind="ExternalInput")
    xb_d = nc.dram_tensor("xb_in", [BPC, T, H], BF16, kind="ExternalInput")
    wv2_d = nc.dram_tensor("wv2_in", [HC, 128, 2, E], F8, kind="ExternalInput")
    wb_d = nc.dram_tensor("wb_in", [HC, 128, S], BF16, kind="ExternalInput")
    wu2_d = nc.dram_tensor("wu2_in", [HC, 128, 2, E], F8, kind="ExternalInput")
    wo2_d = nc.dram_tensor("wo2_in", [EC, 128, 2, H], F8, kind="ExternalInput")
    biasT_d = nc.dram_tensor("biasT_in", [T, T], BF16, kind="ExternalInput")
    ropeC_d = nc.dram_tensor("ropeC_in", [S, T], BF16, kind="ExternalInput")
    ropeS_d = nc.dram_tensor("ropeS_in", [S, T], BF16, kind="ExternalInput")
    gb_d = nc.dram_tensor("gb_in", [S, 4], F32, kind="ExternalInput")
    ubu_d = nc.dram_tensor("ubu_in", [128, EC], F32, kind="ExternalInput")
    ubb_d = nc.dram_tensor("ubb_in", [S, 1], F32, kind="ExternalInput")
    vb_d = nc.dram_tensor("vb_in", [1, E], F32R, kind="ExternalInput")
    y_d = nc.dram_tensor("y_out", [BPC, T, H], F32, kind="ExternalOutput")

    C_UV = 1.0 / (S_X * S_WUV)   # uv projection PSUM descale
    C_O = 1.0 / (S_G * S_WO)     # o projection PSUM descale
    # aT tiles carry S_K * attn; descale rides the gate stt scalar
    S_GK = S_G / S_K

    with tile.TileContext(nc) as tc, ExitStack() as ctx:
        consts = ctx.enter_context(tc.tile_pool(name="consts", bufs=1))
        holdp = ctx.enter_context(tc.tile_pool(name="holdp", bufs=1))
        p12w = ctx.enter_context(tc.tile_pool(name="p12w", bufs=1))
        vp = ctx.enter_context(tc.tile_pool(name="vp", bufs=2 * BPC))
        ktp = ctx.enter_context(tc.tile_pool(name="ktp", bufs=2 * BPC))
        dram = ctx.enter_context(tc.tile_pool(name="dram", bufs=1, space="DRAM"))
        # xn8 spill: fp8 word planes, [lo, hi] order
        xn8_spill = dram.tile([BPC, HC, 2, 128, T], F8)

        # ================= PHASE 1 =================
        with ExitStack() as p1:
            xp = p1.enter_context(tc.tile_pool(name="xp", bufs=2 * TC))
            xnp = p1.enter_context(tc.tile_pool(name="xnp", bufs=TC + 1))
            p1w = p1.enter_context(tc.tile_pool(name="p1w", bufs=1))
            xnTp = p1.enter_context(tc.tile_pool(name="xnTp", bufs=2))
            xn8p = p1.enter_context(tc.tile_pool(name="xn8p", bufs=2))
            rw = p1.enter_context(tc.tile_pool(name="rw", bufs=2))
            p1c = p1.enter_context(tc.tile_pool(name="p1c", bufs=1))
            ps = p1.enter_context(tc.tile_pool(name="ps", bufs=4, space="PSUM"))
            psa = p1.enter_context(tc.tile_pool(name="psa", bufs=2, space="PSUM"))
            pstp = p1.enter_context(tc.tile_pool(name="pstp", bufs=2,
                                                 space="PSUM"))

            # sqrt computes std/S_X directly: sqrt(var/S_X^2 + eps/S_X^2)
            eps_t = p1c.tile([128, 1], F32, tag="eps")
            nc.vector.memset(eps_t, EPS / (S_X * S_X))
            warm = p1c.tile([128, 1], F32, tag="warm")
            nc.scalar.activation(out=warm[:], in_=eps_t[:], func=AF.Silu)
            nc.scalar.activation(out=warm[:], in_=warm[:], func=AF.Sqrt)

            def load_x_ln(e):
                """Load x tiles, LayerNorm; xn stored bf16 scaled by S_X.
                e==0 normalizes per tile (fastest start); later elements
                batch the Sqrt so act-table swaps stay rare."""
                batched = e > 0
                xts = []
                mv4 = rw.tile([128, 2, TC], F32, tag="mv4", bufs=2)
                xn_tiles = []
                for tci in range(TC):
                    xt = xp.tile([128, H], BF16, tag="x")
                    nc.sync.dma_start(
                        out=xt, in_=xb_d[e, tci * 128:(tci + 1) * 128, :])
                    st = rw.tile([128, 2, 6], F32, tag="bnst", bufs=4)
                    xv = xt[:].rearrange("p (g d) -> p g d", g=2)
                    nc.vector.bn_stats(out=st[:, 0, :], in_=xv[:, 0, :])
                    nc.vector.bn_stats(out=st[:, 1, :], in_=xv[:, 1, :])
                    nc.vector.bn_aggr(out=mv4[:, :, tci], in_=st[:])
                    xts.append(xt)
                    if not batched:
                        sstd1 = rw.tile([128, 1], F32, tag="sstd1", bufs=4)
                        nc.scalar.activation(
                            out=sstd1[:], in_=mv4[:, 1, tci:tci + 1],
                            func=AF.Sqrt, bias=eps_t[:],
                            scale=1.0 / (S_X * S_X))
                        rstd1 = rw.tile([128, 1], F32, tag="rstd1", bufs=4)
                        nc.vector.reciprocal(out=rstd1[:], in_=sstd1[:])
                        xn = xnp.tile([128, H], BF16, tag="xn")
                        nc.vector.tensor_scalar(
                            out=xn[:], in0=xt[:],
                            scalar1=mv4[:, 0, tci:tci + 1],
                            scalar2=rstd1[:],
                            op0=OP.subtract, op1=OP.mult)
                        xn_tiles.append(xn)
                if batched:
                    sstd = rw.tile([128, TC], F32, tag="sstd", bufs=2)
                    nc.scalar.activation(
                        out=sstd[:], in_=mv4[:, 1, :], func=AF.Sqrt,
                        bias=eps_t[:], scale=1.0 / (S_X * S_X))
                    rstd = rw.tile([128, TC], F32, tag="rstd", bufs=2)
                    nc.vector.reciprocal(out=rstd[:], in_=sstd[:])
                    for tci in range(TC):
                        xn = xnp.tile([128, H], BF16, tag="xn")
                        nc.vector.tensor_scalar(
                            out=xn[:], in0=xts[tci][:],
                            scalar1=mv4[:, 0, tci:tci + 1],
                            scalar2=rstd[:, tci:tci + 1],
                            op0=OP.subtract, op1=OP.mult)
                        xn_tiles.append(xn)
                return xn_tiles

            def transpose_xn(e, xn_tiles):
                """PE-transpose (bf16) then hi/lo fp8 words (single-op each).
                xnT carries S_X*xn; word order [lo, hi]."""
                xnT = xnTp.tile([128, HC, T], BF16, tag="xnT")
                if e == 0:
                    xn8 = holdp.tile([128, HC, 2, T], F8, tag="xn8_hold",
                                     name="xn8_hold")
                else:
                    xn8 = xn8p.tile([128, HC, 2, T], F8, tag="xn8")
                for tci in range(TC):
                    for hcg in range(2):
                        tps = pstp.tile([128, 4, 128], BF16, tag="tps",
                                        name="tps")
                        for j in range(4):
                            hc = hcg * 4 + j
                            nc.tensor.transpose(
                                tps[:, j, :],
                                xn_tiles[tci][:, hc * 128:(hc + 1) * 128],
                                ident[:])
                        nc.any.tensor_copy(
                            out=xnT[:, hcg * 4:(hcg + 1) * 4,
                                    tci * 128:(tci + 1) * 128],
                            in_=tps[:])
                h4 = HC // 2
                nc.scalar.activation(
                    out=xn8[:, :h4, 1, :], in_=xnT[:, :h4, :], func=AF.Copy)
                nc.vector.tensor_tensor(
                    out=xn8[:, :h4, 0, :], in0=xnT[:, :h4, :],
                    in1=xn8[:, :h4, 1, :], op=OP.subtract)
                nc.gpsimd.tensor_copy(
                    out=xn8[:, h4:, 1, :], in_=xnT[:, h4:, :])
                nc.vector.tensor_tensor(
                    out=xn8[:, h4:, 0, :], in0=xnT[:, h4:, :],
                    in1=xn8[:, h4:, 1, :], op=OP.subtract)
                if e != 0:
                    nc.sync.dma_start(
                        out=xn8_spill[e].rearrange("c j p t -> p c j t"),
                        in_=xn8[:])
                return xnT, xn8

            # ---- priority order: first wv2 pair, x(e0)+LN, rest of wv2 ----
            wv2 = p1w.tile([128, HC, 2, E], F8, tag="wv2", name="wv2")
            for k in range(2):
                nc.sync.dma_start(out=wv2[:, k, :, :], in_=wv2_d[k])
            xn0 = load_x_ln(0)
            for k in range(2, HC):
                nc.sync.dma_start(out=wv2[:, k, :, :], in_=wv2_d[k])

            ident_f = p1c.tile([128, 128], F32, tag="ident_f")
            make_identity(nc, ident_f)
            ident = p1c.tile([128, 128], BF16, tag="ident")
            nc.vector.tensor_copy(out=ident[:], in_=ident_f[:])
            perm_f = p1c.tile([128, 128], F32, tag="perm_f")
            nc.gpsimd.memset(perm_f, 0.0)
            for base in (-64, 64):
                nc.gpsimd.affine_select(
                    out=perm_f, in_=perm_f, compare_op=OP.not_equal,
                    fill=1.0, base=base, pattern=[[-1, 128]],
                    channel_multiplier=1)
            perm = p1c.tile([128, 128], BF16, tag="perm")
            nc.vector.tensor_copy(out=perm[:], in_=perm_f[:])
            # base weights (bf16)
            wb = p1w.tile([128, HC, S], BF16, tag="wb", name="wb")
            nc.sync.dma_start(
                out=wb, in_=wb_d[:].rearrange("c p s -> p c s"))
            gb = p1c.tile([S, 4], F32, tag="gb")
            nc.sync.dma_start(out=gb, in_=gb_d[:])
            ubu = consts.tile([128, EC], F32, tag="ubu")
            nc.sync.dma_start(out=ubu, in_=ubu_d[:])
            ubb = p1c.tile([S, 1], F32, tag="ubb")
            nc.sync.dma_start(out=ubb, in_=ubb_d[:])
            if with_vbias:
                of = p1c.tile([1, 128], F32, tag="ones_row_f")
                nc.vector.memset(of, 1.0)
                ones_row = p1c.tile([1, 128], F32R, tag="ones_row")
                nc.vector.tensor_copy(out=ones_row[:], in_=of[:])
                vb_row = p1c.tile([1, E], F32R, tag="vb_row")
                nc.sync.dma_start(out=vb_row, in_=vb_d[:])

            ropeC = p1c.tile([S, T], BF16, tag="ropeC")
            nc.sync.dma_start(out=ropeC, in_=ropeC_d[:])
            ropeS = p1c.tile([S, T], BF16, tag="ropeS")
            nc.sync.dma_start(out=ropeS, in_=ropeS_d[:])
            biasT = p1c.tile([128, TC, T], BF16, tag="biasT")
            nc.sync.dma_start(
                out=biasT, in_=biasT_d[:].rearrange("(c p) i -> p c i", p=128))

            # u-projection weights preloaded during phase 1 (kills the
            # phase-boundary DMA bubble)
            wu2 = p12w.tile([128, HC, 2, E], F8, tag="wu2", name="wu2")
            for k in range(HC):
                nc.sync.dma_start(out=wu2[:, k, :, :], in_=wu2_d[k])

            def rope_pre(bps):
                # bps carries S_X * base_pre -> descale via silu scale
                ubT = rw.tile([S, T], BF16, tag="ubT")
                nc.scalar.activation(
                    out=ubT[:], in_=bps[:], func=AF.Silu, bias=ubb[:],
                    scale=1.0 / S_X)
                pres = []
                for qi in (0, 1):
                    pre = rw.tile([S, T], BF16, tag="pre")
                    nc.vector.tensor_scalar(
                        out=pre[:], in0=ubT[:],
                        scalar1=gb[:, 2 * qi:2 * qi + 1],
                        scalar2=gb[:, 2 * qi + 1:2 * qi + 2],
                        op0=OP.mult, op1=OP.add)
                    pres.append(pre)
                return pres

            def rope_finish(pres):
                qkts = []
                for pre in pres:
                    sps = ps.tile([128, T], F32, tag="ps", name="sps")
                    nc.tensor.matmul(sps[:], perm[:], pre[:],
                                     start=True, stop=True)
                    t1 = rw.tile([S, T], BF16, tag="t1")
                    nc.vector.tensor_tensor(
                        out=t1[:], in0=pre[:], in1=ropeC[:], op=OP.mult)
                    t2 = rw.tile([S, T], BF16, tag="t2")
                    nc.vector.tensor_tensor(
                        out=t2[:], in0=sps[:], in1=ropeS[:], op=OP.mult)
                    qkt = rw.tile([S, T], BF16, tag="qkt", bufs=2)
                    nc.vector.tensor_tensor(
                        out=qkt[:], in0=t1[:], in1=t2[:], op=OP.add)
                    qkts.append(qkt)
                return qkts

            def base_proj(xnT):
                bps = ps.tile([128, T], F32, tag="ps", name="bps")
                for k in range(HC):
                    nc.tensor.matmul(
                        bps[:], wb[:, k, :], xnT[:, k, :],
                        start=(k == 0), stop=(k == HC - 1))
                return bps

            def scores_kern(qT, kT):
                """q@k^T (f32r) then kern hi/lo fp8 (pair tiles [128,2,2,T])."""
                khl_tiles = [ktp.tile([128, 2, 2, T], F8, tag="khl",
                                      name="khl")
                             for _ in range(TC // 2)]
                for jcp in range(TC // 2):
                    kf = rw.tile([128, 2, T], BF16, tag="kernf", bufs=2)
                    for j in range(2):
                        jc = 2 * jcp + j
                        scps = ps.tile([128, T], F32, tag="ps", name="scps")
                        nc.tensor.matmul(
                            scps[:], kT[:, jc * 128:(jc + 1) * 128], qT[:],
                            start=True, stop=True)
                        tadd = rw.tile([128, T], F32, tag="tadd", bufs=2)
                        nc.vector.tensor_tensor(
                            out=tadd[:], in0=scps[:], in1=biasT[:, jc, :],
                            op=OP.add)
                        # relu(t)^2 == max(t,0)*t (t pre-scaled by sqrt(S_K))
                        nc.vector.scalar_tensor_tensor(
                            out=kf[:, j, :], in0=tadd[:], scalar=0.0,
                            in1=tadd[:], op0=OP.max, op1=OP.mult)
                    khl = khl_tiles[jcp]
                    nc.gpsimd.tensor_copy(out=khl[:, :, 0, :], in_=kf[:])
                    nc.vector.tensor_tensor(
                        out=khl[:, :, 1, :], in0=kf[:], in1=khl[:, :, 0, :],
                        op=OP.subtract)
                return khl_tiles

            def v_fin(v2_tiles, tci, fs, bank):
                if with_vbias:
                    nc.tensor.matmul(
                        bank[:], ones_row[:],
                        vb_row[:, fs * 512:(fs + 1) * 512],
                        start=False, stop=True)
                nc.scalar.activation(
                    out=v2_tiles[tci // 2][:, tci % 2,
                                           fs * 512:(fs + 1) * 512],
                    in_=bank[:], func=AF.Silu, scale=C_UV)

            def v_bank(xn8, tci, fs, bank):
                """3-pass fp8 DR accumulation for one [t128, col512] bank."""
                for c in range(0, HC, 2):
                    nc.tensor.matmul(
                        bank[:],
                        xn8[:, c:c + 2, 1, tci * 128:(tci + 1) * 128],
                        wv2[:, c:c + 2, 0, fs * 512:(fs + 1) * 512],
                        start=(c == 0), stop=False, perf_mode=DR)
                for c in range(HC):
                    nc.tensor.matmul(
                        bank[:],
                        xn8[:, c, 0:2, tci * 128:(tci + 1) * 128],
                        wv2[:, c, 0:2, fs * 512:(fs + 1) * 512],
                        start=False,
                        stop=(c == HC - 1) and not with_vbias,
                        perf_mode=DR)

            def v_proj(xn8):
                v2_tiles = [vp.tile([128, 2, E], F8, tag="v2", name="v2")
                            for _ in range(TC // 2)]
                pairs = [(tci, fs) for tci in range(TC)
                         for fs in range(E // 512)]
                for wi in range(0, len(pairs), 6):
                    wave = pairs[wi:wi + 6]
                    banks = {}
                    for i, p in enumerate(wave):
                        pool_ = psa if i < 2 else ps
                        banks[p] = pool_.tile([128, 512], F32,
                                              tag="psa" if i < 2 else "ps",
                                              name="vbank")
                    for (tci, fs) in wave:
                        v_bank(xn8, tci, fs, banks[(tci, fs)])
                    for (tci, fs) in wave:
                        v_fin(v2_tiles, tci, fs, banks[(tci, fs)])
                return v2_tiles

            xn8_hold = None
            xn_next = xn0
            kv_held = []
            for e in range(BPC):
                xn_e = xn_next
                xnT_e, xn8_e = transpose_xn(e, xn_e)
                if e == 0:
                    xn8_hold = xn8_e
                xn_next = load_x_ln(e + 1) if e + 1 < BPC else None
                bps = base_proj(xnT_e)
                pres = rope_pre(bps)
                qT, kT = rope_finish(pres)
                khl_e = scores_kern(qT, kT)
                v_e = v_proj(xn8_e)
                kv_held.append((khl_e, v_e))

        # ================= PHASE 2 =================
        # u-projection + attn@v + gating + o-projection, fused per element:
        # attn never leaves PSUM (no aT spill round-trip).
        with ExitStack() as p2:
            xn82p = p2.enter_context(tc.tile_pool(name="xn82p", bufs=2))
            p2w = p2.enter_context(tc.tile_pool(name="p2w", bufs=1))
            utp = p2.enter_context(tc.tile_pool(name="utp", bufs=2))
            gfp = p2.enter_context(tc.tile_pool(name="gfp", bufs=2))
            gtp = p2.enter_context(tc.tile_pool(name="gtp", bufs=EC))
            yp = p2.enter_context(tc.tile_pool(name="yp", bufs=2))
            xrp = p2.enter_context(tc.tile_pool(name="xrp", bufs=2))
            psu = p2.enter_context(tc.tile_pool(name="psu", bufs=3,
                                                space="PSUM"))
            psv = p2.enter_context(tc.tile_pool(name="psv", bufs=2,
                                                space="PSUM"))
            pso = p2.enter_context(tc.tile_pool(name="pso", bufs=3,
                                                space="PSUM"))

            def load_xn82(e):
                xn82 = xn82p.tile([128, HC, 2, T], F8, tag="xn82")
                nc.sync.dma_start(
                    out=xn82, in_=xn8_spill[e].rearrange("c j p t -> p c j t"))
                return xn82

            wo2 = [None] * (EC // 2)

            def load_wo_pair(ecp):
                woc = p2w.tile([128, 2, 2, H], F8, tag=f"wo{ecp}", name="woc")
                for i in range(2):
                    nc.sync.dma_start(
                        out=woc[:, i, :, :], in_=wo2_d[2 * ecp + i])
                wo2[ecp] = woc

            def u_bank(xn82, ec, bank):
                """3-pass fp8 DR for one [e128, T] u-projection bank."""
                for c in range(0, HC, 2):
                    nc.tensor.matmul(
                        bank[:],
                        wu2[:, c:c + 2, 0, ec * 128:(ec + 1) * 128],
                        xn82[:, c:c + 2, 1, :],
                        start=(c == 0), stop=False, perf_mode=DR)
                for c in range(HC):
                    nc.tensor.matmul(
                        bank[:],
                        wu2[:, c, 0:2, ec * 128:(ec + 1) * 128],
                        xn82[:, c, 0:2, :],
                        start=False, stop=(c == HC - 1), perf_mode=DR)

            def attnv_bank(khl_tiles, v2_tiles, ec, bank):
                """2-pass fp8 DR attention for one [e128, T] bank."""
                for w_ in range(2):       # 0 = hi planes, 1 = lo planes
                    for jcp in range(TC // 2):
                        nc.tensor.matmul(
                            bank[:],
                            v2_tiles[jcp][:, :, ec * 128:(ec + 1) * 128],
                            khl_tiles[jcp][:, :, w_, :],
                            start=(w_ == 0 and jcp == 0),
                            stop=(w_ == 1 and jcp == TC // 2 - 1),
                            perf_mode=DR)

            def u_attn_gate(e, xn82, khl_tiles, v2_tiles, chase_wo):
                """Per ec: u-proj bank + attnv bank, silu + gate from PSUM.
                g8 pair tiles [128, 2(ec), 2(lo/hi), T] fp8."""
                g8_tiles = [gtp.tile([128, 2, 2, T], F8, tag="g8", name="g8")
                            for _ in range(EC // 2)]
                gf = None
                for ec in range(EC):
                    ub = psu.tile([128, T], F32, tag="psu", name="ub")
                    u_bank(xn82, ec, ub)
                    ab = psv.tile([128, T], F32, tag="psv", name="ab")
                    attnv_bank(khl_tiles, v2_tiles, ec, ab)
                    if ec % 2 == 0:
                        gf = gfp.tile([128, 2, T], FP16, tag="gf")
                    ut = utp.tile([128, T], BF16, tag="uT")
                    nc.scalar.activation(
                        out=ut[:], in_=ub[:], func=AF.Silu,
                        bias=ubu[:, ec:ec + 1], scale=C_UV)
                    nc.vector.scalar_tensor_tensor(
                        out=gf[:, ec % 2, :], in0=ut[:], scalar=S_GK,
                        in1=ab[:], op0=OP.mult, op1=OP.mult)
                    if ec % 2 == 1:
                        ecp = ec // 2
                        g8 = g8_tiles[ecp]
                        nc.scalar.activation(
                            out=g8[:, :, 1, :], in_=gf[:], func=AF.Copy)
                        nc.vector.tensor_tensor(
                            out=g8[:, :, 0, :], in0=gf[:], in1=g8[:, :, 1, :],
                            op=OP.subtract)
                        if chase_wo and ecp < EC // 2:
                            load_wo_pair(ecp)
                return g8_tiles

            def o_bank(g8_tiles, tci, hs, bank):
                """3-pass fp8 DR for one [t128, h512] o-projection bank."""
                t0 = tci * 128
                h0 = hs * 512
                for ecp in range(EC // 2):
                    nc.tensor.matmul(
                        bank[:],
                        g8_tiles[ecp][:, :, 1, t0:t0 + 128],
                        wo2[ecp][:, :, 0, h0:h0 + 512],
                        start=(ecp == 0), stop=False, perf_mode=DR)
                for ec in range(EC):
                    nc.tensor.matmul(
                        bank[:],
                        g8_tiles[ec // 2][:, ec % 2, 0:2, t0:t0 + 128],
                        wo2[ec // 2][:, ec % 2, 0:2, h0:h0 + 512],
                        start=False, stop=(ec == EC - 1), perf_mode=DR)

            def o_fin(e, tci, hs, bank):
                xr = xrp.tile([128, 512], F32, tag="xr")
                nc.sync.dma_start(
                    out=xr,
                    in_=x_d[e, tci * 128:(tci + 1) * 128,
                            hs * 512:(hs + 1) * 512])
                yt = yp.tile([128, 512], F32, tag="y")
                nc.vector.scalar_tensor_tensor(
                    out=yt[:], in0=bank[:], scalar=C_O, in1=xr[:],
                    op0=OP.mult, op1=OP.add)
                nc.sync.dma_start(
                    out=y_d[e, tci * 128:(tci + 1) * 128,
                            hs * 512:(hs + 1) * 512],
                    in_=yt[:])

            def o_bank_half(g8_tiles, tci, hs, bank, half):
                """Half of the 3-pass accumulation (ec chunks split 2-way)."""
                t0 = tci * 128
                h0 = hs * 512
                ecps = range(half * (EC // 4), (half + 1) * (EC // 4))
                for i, ecp in enumerate(ecps):
                    nc.tensor.matmul(
                        bank[:],
                        g8_tiles[ecp][:, :, 1, t0:t0 + 128],
                        wo2[ecp][:, :, 0, h0:h0 + 512],
                        start=(i == 0), stop=False, perf_mode=DR)
                ecs = list(range(half * (EC // 2), (half + 1) * (EC // 2)))
                for i, ec in enumerate(ecs):
                    nc.tensor.matmul(
                        bank[:],
                        g8_tiles[ec // 2][:, ec % 2, 0:2, t0:t0 + 128],
                        wo2[ec // 2][:, ec % 2, 0:2, h0:h0 + 512],
                        start=False, stop=(i == len(ecs) - 1), perf_mode=DR)

            def o_fin2(e, tci, hs, b0, b1):
                xr = xrp.tile([128, 512], F32, tag="xr")
                nc.sync.dma_start(
                    out=xr,
                    in_=x_d[e, tci * 128:(tci + 1) * 128,
                            hs * 512:(hs + 1) * 512])
                tsum = yp.tile([128, 512], F32, tag="tsum")
                nc.vector.scalar_tensor_tensor(
                    out=tsum[:], in0=b0[:], scalar=C_O, in1=xr[:],
                    op0=OP.mult, op1=OP.add)
                yt = yp.tile([128, 512], F32, tag="y")
                nc.vector.scalar_tensor_tensor(
                    out=yt[:], in0=b1[:], scalar=C_O, in1=tsum[:],
                    op0=OP.mult, op1=OP.add)
                nc.sync.dma_start(
                    out=y_d[e, tci * 128:(tci + 1) * 128,
                            hs * 512:(hs + 1) * 512],
                    in_=yt[:])

            def o_proj(e, g8_tiles):
                pairs8 = [(tci, hs) for tci in range(TC)
                          for hs in range(H // 512)]
                if e == BPC - 1:
                    # tail: two parallel half-chains per output so the final
                    # drain is half as long
                    for wi in range(0, len(pairs8), 2):
                        wave = pairs8[wi:wi + 2]
                        hb = {}
                        for j, p in enumerate(wave):
                            hb[p] = (pso.tile([128, 512], F32, tag="pso",
                                              name="ob0"),
                                     psu.tile([128, 512], F32, tag="psu",
                                              name="ob1"))
                        for (tci, hs) in wave:
                            o_bank_half(g8_tiles, tci, hs,
                                        hb[(tci, hs)][0], 0)
                            o_bank_half(g8_tiles, tci, hs,
                                        hb[(tci, hs)][1], 1)
                        for (tci, hs) in wave:
                            o_fin2(e, tci, hs, *hb[(tci, hs)])
                else:
                    for wi in range(0, len(pairs8), 3):
                        wave = pairs8[wi:wi + 3]
                        banks = {p: pso.tile([128, 512], F32, tag="pso",
                                             name="obank") for p in wave}
                        for (tci, hs) in wave:
                            o_bank(g8_tiles, tci, hs, banks[(tci, hs)])
                        for (tci, hs) in wave:
                            o_fin(e, tci, hs, banks[(tci, hs)])

            xn82_next = None
            for e in range(BPC):
                if e == 0:
                    xn82 = xn8_hold
                else:
                    xn82 = xn82_next
                khl_e, v_e = kv_held[e]
                g8_tiles = u_attn_gate(e, xn82, khl_e, v_e,
                                       chase_wo=(e == 0))
                if e + 1 < BPC:
                    xn82_next = load_xn82(e + 1)
                o_proj(e, g8_tiles)

    return nc


_BUILD_CACHE = {}


def _get_nc(with_vbias):
    key = bool(with_vbias)
    if key not in _BUILD_CACHE:
        nc = bacc.Bacc("TRN2", target_bir_lowering=False)
        _emit(nc, with_vbias)
        nc.compile()
        _BUILD_CACHE[key] = nc
    return _BUILD_CACHE[key]


def _rope_tables():
    """Rope sin/cos tables, computed with jax-on-cpu float32 ops exactly as
    the reference does (sin/cos of large fp32 arguments are implementation-
    sensitive, so matching op-for-op matters)."""
    import jax
    import jax.numpy as jnp

    cpu = jax.devices("cpu")[0]
    with jax.default_device(cpu):
        half = S // 2
        pos = jnp.arange(T, dtype=jnp.float32)
        inv_freq = 10000.0 ** (jnp.arange(half, dtype=jnp.float32) / half)
        sinusoid = pos[:, None] * inv_freq[None, :]          # [T, half]
        sin = np.asarray(jnp.sin(sinusoid)).astype(np.float32)
        cos = np.asarray(jnp.cos(sinusoid)).astype(np.float32)
    C = np.empty((S, T), np.float32)
    Sg = np.empty((S, T), np.float32)
    C[:half] = cos.T
    C[half:] = cos.T
    Sg[:half] = -sin.T   # q[s<64] = pre[s]*cos - pre[s+64]*sin
    Sg[half:] = sin.T    # q[s>=64] = pre[s]*cos + pre[s-64]*sin
    return C, Sg


def _split8(w, s):
    """2-word e4m3 split at common scale s: returns (hi, lo) planes."""
    hi = (w * s).astype(E4NP)
    lo = ((w * s).astype(np.float32) - hi.astype(np.float32)).astype(E4NP)
    return hi, lo


def _host_prep(x, ln_w, ln_b, uv_w, uv_b, gamma, beta, w, o_w, o_b):
    w_eff = uv_w * ln_w[None, :]                 # fold ln scale into weights
    uvb_eff = uv_b + uv_w @ ln_b                 # fold ln shift into biases
    uv_wT = np.ascontiguousarray(w_eff.T)        # [H, 2E+S]
    w_u = uv_wT[:, :E]
    w_v = uv_wT[:, E:2 * E]
    w_base = uv_wT[:, 2 * E:]
    wo = np.ascontiguousarray(o_w.T)             # [E, H]

    # fp8 hi/lo planes, [hi, lo] word order, chunked layouts
    vh, vl = _split8(w_v, S_WUV)
    wv2 = np.stack([vh.reshape(HC, 128, E), vl.reshape(HC, 128, E)], axis=2)
    uh, ul = _split8(w_u, S_WUV)
    wu2 = np.stack([uh.reshape(HC, 128, E), ul.reshape(HC, 128, E)], axis=2)
    oh, ol = _split8(wo, S_WO)
    wo2 = np.stack([oh.reshape(EC, 128, H), ol.reshape(EC, 128, H)], axis=2)

    wb = np.ascontiguousarray(w_base.reshape(HC, 128, S)).astype(BFNP)

    idx = np.arange(T)
    sqrt_sk = np.float32(np.sqrt(S_K))
    biasT = np.ascontiguousarray(
        w[idx[:, None] - idx[None, :] + (L - 1)] * sqrt_sk).astype(BFNP)

    ropeC, ropeS = _rope_tables()

    inv_sqrt_s = np.float32(1.0 / np.sqrt(np.float32(S)))
    # q side picks up 1/sqrt(S) and sqrt(S_K) so the DVE adds stay fused
    gb = np.stack([gamma[0] * inv_sqrt_s * sqrt_sk,
                   beta[0] * inv_sqrt_s * sqrt_sk,
                   gamma[1], beta[1]], axis=1).astype(np.float32)

    ubu = np.ascontiguousarray(
        uvb_eff[:E].reshape(EC, 128).T).astype(np.float32)
    ubb = uvb_eff[2 * E:].reshape(S, 1).astype(np.float32)
    vb = (uvb_eff[E:2 * E].reshape(1, E) * np.float32(S_X * S_WUV)
          ).astype(np.float32)
    return {
        "wv2_in": np.ascontiguousarray(wv2), "wb_in": wb,
        "wu2_in": np.ascontiguousarray(wu2),
        "wo2_in": np.ascontiguousarray(wo2), "biasT_in": biasT,
        "ropeC_in": ropeC.astype(BFNP), "ropeS_in": ropeS.astype(BFNP), "gb_in": gb,
        "ubu_in": ubu, "ubb_in": ubb, "vb_in": vb,
    }


def kernel(x, ln_w, ln_b, uv_w, uv_b, gamma, beta, w, o_w, o_b):
    x = np.ascontiguousarray(np.asarray(x, dtype=np.float32))
    args = [np.asarray(a, np.float32) for a in
            (ln_w, ln_b, uv_w, uv_b, gamma, beta, w, o_w, o_b)]
    ln_w, ln_b, uv_w, uv_b, gamma, beta, w, o_w, o_b = args

    shared = _host_prep(x, ln_w, ln_b, uv_w, uv_b, gamma, beta, w, o_w, o_b)
    with_vbias = bool(np.any(shared["vb_in"]))
    nc = _get_nc(with_vbias)

    in_maps = []
    for c in range(NCORES):
        m = dict(shared)
        xs = np.ascontiguousarray(x[c * BPC:(c + 1) * BPC])
        m["x_in"] = xs
        m["xb_in"] = xs.astype(BFNP)
        in_maps.append(m)

    res = run_bass_kernel_spmd(nc, in_maps, core_ids=list(range(NCORES)))
    out = np.concatenate([r["y_out"] for r in res.results], axis=0)
    if np.any(o_b):
        out = out + o_b[None, None, :]
    return out
